# revision 1
# baseline (speedup 1.0000x reference)
"""LSTM layer kernel for Trainium2 (8 NeuronCores).

Strategy: tensor-parallel over the gate dimension (4H=4096 -> 512 gate rows
per core, covering 128 hidden channels x 4 gates, reordered [i|f|o|g]).

Phase 1 (projection): xg = x @ W_ih_j.T + bias, batched over (t, b) rows,
written to internal DRAM in [S, B, 512] layout (t-major row tiles).

Phase 2 (scan): per timestep
  - PSUM init: gates = I32.T @ xg_t  (adds the precomputed input projection)
  - gates += h.T_k.T @ W_hh_j.T_k  for 8 K-tiles (W resident in SBUF)
  - sigmoid on [i|f|o], tanh on g (ScalarE), c/h update (VectorE)
  - h_j [32,128] -> PE transpose -> [128,32] -> DRAM bounce
  - AllGather across 8 cores -> h.T [1024, 32] -> SBUF for next step
"""

import numpy as np

import concourse.bass as bass
import concourse.bacc as bacc
import concourse.mybir as mybir
from concourse import tile
from concourse.bass_utils import run_bass_kernel_spmd

F32 = mybir.dt.float32
AF = mybir.ActivationFunctionType
ALU = mybir.AluOpType

B = 32          # batch
I = 1024        # input size
H = 1024        # hidden size
NCORES = 8
GC = 4 * H // NCORES    # 512 gate rows per core
HC = H // NCORES        # 128 hidden channels per core
KI = I // 128           # 8 input K-tiles
KH = H // 128           # 8 hidden K-tiles

_CACHE = {}


def build_nc(S):
    nc = bacc.Bacc(
        "TRN2", target_bir_lowering=False, debug=False, num_devices=NCORES
    )
    BT = B * S
    MT = BT // 128          # projection M-tiles (t-major: tile m = t in [4m,4m+4))

    xT = nc.dram_tensor("xT", [I, BT], F32, kind="ExternalInput").ap()
    wihT = nc.dram_tensor("wihT", [I, GC], F32, kind="ExternalInput").ap()
    whhT = nc.dram_tensor("whhT", [H, GC], F32, kind="ExternalInput").ap()
    bias_bc = nc.dram_tensor("bias_bc", [128, GC], F32, kind="ExternalInput").ap()
    h0T = nc.dram_tensor("h0T", [H, B], F32, kind="ExternalInput").ap()
    c0j = nc.dram_tensor("c0j", [B, HC], F32, kind="ExternalInput").ap()
    eye = nc.dram_tensor("eye32", [B, B], F32, kind="ExternalInput").ap()

    outj = nc.dram_tensor("outj", [B, S, HC], F32, kind="ExternalOutput").ap()
    clastj = nc.dram_tensor("clastj", [B, HC], F32, kind="ExternalOutput").ap()

    xg = nc.dram_tensor("xg", [S, B, GC], F32).ap()  # internal scratch

    with tile.TileContext(nc) as tc:
        with (
            tc.tile_pool(name="const", bufs=1) as constp,
            tc.tile_pool(name="plhs", bufs=3) as plhs,
            tc.tile_pool(name="pout", bufs=3) as poutp,
            tc.tile_pool(name="psum", bufs=2, space="PSUM") as psump,
            tc.tile_pool(name="psumT", bufs=2, space="PSUM") as psumTp,
            tc.tile_pool(name="xgp", bufs=4) as xgp,
            tc.tile_pool(name="hp", bufs=2) as hp,
            tc.tile_pool(name="state", bufs=2) as statep,
            tc.tile_pool(name="act", bufs=2) as actp,
            tc.tile_pool(name="dram", bufs=2, space="DRAM") as dramp,
        ):
            # ---- resident constants ----
            eye_sb = constp.tile([B, B], F32)
            nc.sync.dma_start(eye_sb, eye)
            bias_sb = constp.tile([128, GC], F32)
            nc.sync.dma_start(bias_sb, bias_bc)
            wih_sb = constp.tile([128, KI * GC], F32)  # k-tile = [:, GC*k:GC*(k+1)]
            nc.sync.dma_start(
                wih_sb.rearrange("p (k g) -> p k g", k=KI),
                wihT.rearrange("(k p) g -> p k g", p=128),
            )
            whh_sb = constp.tile([128, KH * GC], F32)
            nc.sync.dma_start(
                whh_sb.rearrange("p (k g) -> p k g", k=KH),
                whhT.rearrange("(k p) g -> p k g", p=128),
            )

            # ---- phase 1: input projection ----
            for m in range(MT):
                lhs = plhs.tile([128, KI * 128], F32)  # xT[:, 128m:128(m+1)]
                nc.sync.dma_start(
                    lhs.rearrange("p (k c) -> p k c", k=KI),
                    xT[:, 128 * m : 128 * (m + 1)].rearrange(
                        "(k p) c -> p k c", p=128
                    ),
                )
                ps = psump.tile([128, GC], F32)
                for k in range(KI):
                    nc.tensor.matmul(
                        ps,
                        lhs[:, 128 * k : 128 * (k + 1)],
                        wih_sb[:, GC * k : GC * (k + 1)],
                        start=(k == 0),
                        stop=(k == KI - 1),
                    )
                ob = poutp.tile([128, GC], F32)
                nc.vector.tensor_add(ob, ps, bias_sb)
                nc.sync.dma_start(
                    xg[4 * m : 4 * (m + 1)].rearrange("t b g -> (t b) g"), ob
                )

            # ---- phase 2: recurrent scan ----
            h_sb = hp.tile([128, KH * B], F32)  # h.T staged; k-tile = [:, B*k:B*(k+1)]
            nc.sync.dma_start(
                h_sb.rearrange("p (k b) -> p k b", k=KH),
                h0T.rearrange("(k p) b -> p k b", p=128),
            )
            c_old = statep.tile([B, HC], F32)
            nc.sync.dma_start(c_old, c0j)

            for t in range(S):
                xg_t = xgp.tile([B, GC], F32)
                nc.sync.dma_start(xg_t, xg[t])

                ps = psump.tile([B, GC], F32, tag="scan_ps")
                nc.tensor.matmul(ps, eye_sb, xg_t, start=True, stop=False)
                for k in range(KH):
                    nc.tensor.matmul(
                        ps,
                        h_sb[:, B * k : B * (k + 1)],
                        whh_sb[:, GC * k : GC * (k + 1)],
                        start=False,
                        stop=(k == KH - 1),
                    )

                sig = actp.tile([B, 3 * HC], F32)
                nc.scalar.activation(sig, ps[:, 0 : 3 * HC], AF.Sigmoid)
                tg = actp.tile([B, HC], F32)
                nc.scalar.activation(tg, ps[:, 3 * HC : 4 * HC], AF.Tanh)

                ig = actp.tile([B, HC], F32)
                nc.vector.tensor_mul(ig, sig[:, 0:HC], tg)
                cf = actp.tile([B, HC], F32)
                nc.vector.tensor_mul(cf, sig[:, HC : 2 * HC], c_old)
                c_new = statep.tile([B, HC], F32, tag="c")
                nc.vector.tensor_add(c_new, ig, cf)

                tch = actp.tile([B, HC], F32)
                nc.scalar.activation(tch, c_new, AF.Tanh)
                h_b = actp.tile([B, HC], F32)
                nc.vector.tensor_mul(h_b, sig[:, 2 * HC : 3 * HC], tch)

                nc.sync.dma_start(outj[:, t, :], h_b)

                # h -> h.T -> DRAM -> AllGather -> SBUF (next step's lhsT)
                psT = psumTp.tile([HC, B], F32)
                nc.tensor.transpose(psT, h_b, eye_sb)
                hT_sb = actp.tile([HC, B], F32)
                nc.vector.tensor_copy(hT_sb, psT)
                ag_in = dramp.tile([HC, B], F32)
                nc.sync.dma_start(ag_in, hT_sb)
                ag_out = dramp.tile([H, B], F32)
                nc.gpsimd.collective_compute(
                    "AllGather",
                    ALU.bypass,
                    replica_groups=[list(range(NCORES))],
                    ins=[ag_in.opt()],
                    outs=[ag_out.opt()],
                )
                h_sb = hp.tile([128, KH * B], F32, tag="h_sb")
                nc.sync.dma_start(
                    h_sb.rearrange("p (k b) -> p k b", k=KH),
                    ag_out.rearrange("(k p) b -> p k b", p=128),
                )
                c_old = c_new

            nc.sync.dma_start(clastj, c_old)

    nc.compile()
    return nc


def _prep_inputs(input_seq, h0, c0, weight_ih, bias_ih, weight_hh, bias_hh):
    """Host-side sharding: returns per-core input maps."""
    S = input_seq.shape[1]
    f32 = np.float32

    # t-major flattened transpose: xT[i, t*B + b]
    xT = np.ascontiguousarray(
        np.asarray(input_seq, dtype=f32).transpose(2, 1, 0).reshape(I, S * B)
    )
    h0T = np.ascontiguousarray(np.asarray(h0, dtype=f32).T)
    eye = np.eye(B, dtype=f32)

    def reorder_rows(w):
        # [4H, ...] in (i, f, g, o) blocks -> per-core [i_j|f_j|o_j|g_j]
        wi, wf, wg, wo = np.split(np.asarray(w, dtype=f32), 4, axis=0)
        out = []
        for j in range(NCORES):
            sl = slice(j * HC, (j + 1) * HC)
            out.append(np.concatenate([wi[sl], wf[sl], wo[sl], wg[sl]], axis=0))
        return out

    wih_parts = reorder_rows(weight_ih)
    whh_parts = reorder_rows(weight_hh)
    bias_parts = reorder_rows((np.asarray(bias_ih) + np.asarray(bias_hh))[:, None])

    in_maps = []
    for j in range(NCORES):
        in_maps.append(
            {
                "xT": xT,
                "wihT": np.ascontiguousarray(wih_parts[j].T),
                "whhT": np.ascontiguousarray(whh_parts[j].T),
                "bias_bc": np.ascontiguousarray(
                    np.broadcast_to(bias_parts[j].reshape(1, GC), (128, GC))
                ),
                "h0T": h0T,
                "c0j": np.ascontiguousarray(
                    np.asarray(c0, dtype=f32)[:, j * HC : (j + 1) * HC]
                ),
                "eye32": eye,
            }
        )
    return in_maps


def kernel(input_seq, h0, c0, weight_ih, bias_ih, weight_hh, bias_hh):
    S = input_seq.shape[1]
    if S not in _CACHE:
        _CACHE[S] = build_nc(S)
    nc = _CACHE[S]

    in_maps = _prep_inputs(
        input_seq, h0, c0, weight_ih, bias_ih, weight_hh, bias_hh
    )
    res = run_bass_kernel_spmd(nc, in_maps, list(range(NCORES)))

    outs = [res.results[j]["outj"] for j in range(NCORES)]
    clasts = [res.results[j]["clastj"] for j in range(NCORES)]
    output_seq = np.concatenate(outs, axis=2).astype(np.float32)
    c_last = np.concatenate(clasts, axis=1).astype(np.float32)
    h_last = np.ascontiguousarray(output_seq[:, -1, :])
    return output_seq, h_last, c_last
